# revision 33
# baseline (speedup 1.0000x reference)
"""Trainium2 Bass kernel for nn_AttentionSimilarity.

Contract: kernel(**inputs) takes the FULL unsharded inputs (numpy) and
returns the FULL [64, 64] similarity matrix, distributing work across 8
NeuronCores internally.

Structure:
  prog1 (projections, sharded by batch): each core projects its 8
    a-batches and 8 b-batches through the three two-layer MLPs using
    fp8e4m3 DoubleRowSwInterleave matmuls (two 128-deep contraction
    chunks fused per PE instruction at 0.5 cycles/col), emitting
    q/k/v chunks in [inner, (batch, n)] fp16 layout. Host gathers the
    a-side to full tensors.
  prog2 (attention, sharded by p = b-side batch): per core, both
    attention paths for its 8 p's against all 64 q's. All matmuls are
    fp8 DoubleRowSwInterleave: scores pair (valid, zero) halves;
    aligned pairs the (vaL, vaR) two-plane trick; the dot/norm
    reductions over the inner dim use sliding-window one-hot selector
    weights paired (SQ, M) so one PE pass accumulates both the cosine
    numerator and the norm into a single PSUM accumulator.

Math notes:
  - softmax feeds only cosine similarity, which is scale-invariant in
    the aligned vector, so the softmax max-shift and denominator cancel:
    softmax reduces to exp(scores/8 + bias) for any constant bias.
  - va/vb are pre-scaled by 1/VSCALE so aligned values and their squares
    stay in fp8e4m3 range; the scale cancels exactly in dot/|y|.
  - 1/max(|y|, eps) is computed as exp(-0.5 * ln(ny^2 + eps^2)).

DoubleRowSwInterleave weight packing (validated on HW): for logical
W0, W1 [K=128, M=128], the SBUF buffer holds
  buf[k, 2u] = W0[k, 127-u], buf[k, 2u+1] = W1[k, 127-u]
and out = W0^T @ X0 + W1^T @ X1 where X0/X1 are the (two, f) halves of
the ifmap access pattern. Selector masters slide their 256-col window
by +2 per target row, so 4 fixed one-hot cells serve every window.
"""

import os
import sys

sys.path.insert(0, "/opt/trn_rl_repo")
os.environ.setdefault("NEURON_RT_RESET_CORES", "1")

import numpy as np
import ml_dtypes

import bass_rust
import concourse.bass as bass
import concourse.mybir as mybir
import concourse.tile as tile
from concourse.bass_utils import run_bass_kernel_spmd

F32 = mybir.dt.float32
BF16 = mybir.dt.bfloat16
F16 = mybir.dt.float16
F8 = mybir.dt.float8e4
NPF8 = ml_dtypes.float8_e4m3
AF = mybir.ActivationFunctionType
DRSWI = mybir.MatmulPerfMode.DoubleRowSwInterleave

B = 64          # batches per side
C = 512         # channels
N = 100         # H*W tokens per batch
INNER = 64      # projected dim
CORES = 8
PB = B // CORES  # batches per core (8)
BN = PB * N      # 800: (batch, n) columns per core chunk
EPS = 1e-8
VSCALE = 64.0    # va/vb pre-scale so As and As^2 fit fp8e4m3
EXP_BIAS = float(os.environ.get("K_EXP_BIAS", "-0.25"))
# of path1's 64 half-units, how many run their SQ/M mul pair on Pool
# (DVE otherwise); path2 mul pairs always run on Pool via
# scalar_tensor_tensor (billed at the higher default GPSIMD efficiency).
POOL1 = int(os.environ.get("K_POOL1", "32"))
POOL2 = int(os.environ.get("K_POOL2", "15"))
ACTSQ2 = int(os.environ.get("K_ACTSQ2", "0"))
SEL_DEPTH = int(os.environ.get("K_SEL_DEPTH", "6"))
E1_BUFS = int(os.environ.get("K_E1_BUFS", "4"))
E2_BUFS = int(os.environ.get("K_E2_BUFS", "2"))
PENDE_D = int(os.environ.get("K_PENDE_D", "2"))
PENDM_D = int(os.environ.get("K_PENDM_D", "2"))
MPOOL_BUFS = int(os.environ.get("K_MPOOL_BUFS", "6"))

CH1 = [(0, 512), (512, 800)]

_waitsplit_ctr = [0]


def _split_multi_waits(nc, max_waits=1):
    """This container's walrus build accepts at most ONE sync wait per
    instruction; Tile attaches several. Move extras onto preceding
    same-engine NoOps (engines are in-order, so semantics hold)."""
    n_split = 0
    for f in nc.m.functions:
        for blk in f.blocks:
            insts = list(blk.instructions)
            new_list = []
            changed = False
            for inst in insts:
                si = inst.sync_info
                waits = list(si.on_wait) if (si is not None and si.on_wait) else []
                if len(waits) > max_waits:
                    for w in waits[:-max_waits]:
                        _waitsplit_ctr[0] += 1
                        nop = mybir.InstNoOp(
                            name=f"I-waitsplit-{_waitsplit_ctr[0]}",
                            engine=inst.engine,
                            ins=[],
                            outs=[],
                            sync_info=bass_rust.SyncInfo(on_wait=[w], on_update=[]),
                        )
                        nc.register_instruction(nop, overwrite=True)
                        new_list.append(nop)
                        n_split += 1
                    si.on_wait = waits[-max_waits:]
                    inst.sync_info = si
                    changed = True
                new_list.append(inst)
            if changed:
                blk.instructions = new_list
    return n_split


def _ilv(W0, W1):
    """Pack logical stationary pair [K, M] x2 -> interleaved-reversed
    [K, 2M] f32 buffer for DoubleRowSwInterleave (cast to fp8 later)."""
    K_, M_ = W0.shape
    buf = np.zeros((K_, 2 * M_), np.float32)
    buf[:, 0::2] = W0[:, ::-1]
    buf[:, 1::2] = W1[:, ::-1]
    return buf


def _act_sq_spread(total, n_act):
    """Evenly spread n_act True flags over `total` slots."""
    return [((u * n_act) // total) != (((u + 1) * n_act) // total)
            for u in range(total)]


def _register_bias_consts(nc, values):
    """Activation float biases need a registered [128, 1] const AP."""
    for v in values:
        if (mybir.dt.float32, v) in nc.const_aps.aps:
            continue
        t = nc.alloc_sbuf_tensor(f"const-float32-{v}", [128, 1], F32)
        nc.gpsimd.memset(t.ap(), v)
        nc.const_aps.aps[(mybir.dt.float32, v)] = t.ap()
    nc.all_engine_barrier()


# ---------------------------------------------------------------- prog1

def build_prog1():
    """Projection program (fp8 DR). Per-core inputs:
      fa_dr, fb_dr [128, 3200] f8: features, DR pair layout:
        col 1600*b + 800*two + n  <->  channel 256*b + 128*two, row k
      w1_dr [128, 6144] f8: per (proj t, pair b, couttile ct) a 256-wide
        interleaved stationary block
      w2_dr [128, 1536] f8: per (t, b) a 256-wide block (out cols 64:128
        zero-padded)
    Outputs: qaT8/kaT8/vaT8/qbT8/kbT8/vbT8 [64, BN] f16.
    """
    nc = bass.Bass("TRN2", target_bir_lowering=False, debug=False,
                   num_devices=CORES)
    fa = nc.dram_tensor("fa_dr", [128, 3200], F8, kind="ExternalInput").ap()
    fb = nc.dram_tensor("fb_dr", [128, 3200], F8, kind="ExternalInput").ap()
    w1 = nc.dram_tensor("w1_dr", [128, 6144], F8, kind="ExternalInput").ap()
    w2 = nc.dram_tensor("w2_dr", [128, 1536], F8, kind="ExternalInput").ap()
    outs = {(s, t): nc.dram_tensor(f"{t}{s}T8", [INNER, BN], F16,
                                   kind="ExternalOutput").ap()
            for s in "ab" for t in "qkv"}

    with tile.TileContext(nc) as tc:
        with (
            tc.tile_pool(name="wpool", bufs=1) as wpool,
            tc.tile_pool(name="fpool", bufs=1) as fpool,
            tc.tile_pool(name="hpool", bufs=2) as hpool,
            tc.tile_pool(name="opool", bufs=2) as opool,
            tc.tile_pool(name="psH", bufs=3, space="PSUM") as psHp,
            tc.tile_pool(name="psO", bufs=1, space="PSUM") as psOp,
        ):
            w1q = wpool.tile([128, 2048], F8, tag="w1q", name="w1q")
            nc.sync.dma_start(w1q[:, 0:1024], w1[:, 0:1024])
            fas = fpool.tile([128, 3200], F8, tag="fa", name="fas")
            nc.sync.dma_start(fas[:, 0:1600], fa[:, 0:1600])
            nc.sync.dma_start(w1q[:, 1024:2048], w1[:, 1024:2048])
            nc.sync.dma_start(fas[:, 1600:3200], fa[:, 1600:3200])
            w1kv = wpool.tile([128, 4096], F8, tag="w1kv", name="w1kv")
            nc.sync.dma_start(w1kv[:], w1[:, 2048:6144])
            w2s = wpool.tile([128, 1536], F8, tag="w2", name="w2s")
            nc.sync.dma_start(w2s[:], w2[:])
            fbs = fpool.tile([128, 3200], F8, tag="fb", name="fbs")
            nc.sync.dma_start(fbs[:], fb[:])

            def w1blk(t, b, ct):
                ti = "qkv".index(t)
                off = ti * 2048 + b * 1024 + ct * 256
                if ti == 0:
                    return w1q[:, off:off + 256]
                return w1kv[:, off - 2048:off - 2048 + 256]

            nop = [0]
            for s, feat in (("a", fas), ("b", fbs)):
                fv = feat[:].rearrange("p (b two f) -> p b two f", b=2, two=2)
                for t in "qkv":
                    hts = [hpool.tile([128, 1600], F8, tag=f"h{pb}",
                                      name=f"h{s}{t}{pb}")
                           for pb in range(2)]
                    for ct in range(4):
                        psH = psHp.tile([128, 800], F32, tag="psH")
                        for b in range(2):
                            for lo, hi in CH1:
                                nc.tensor.matmul(
                                    psH[:, lo:hi], w1blk(t, b, ct),
                                    fv[:, b, :, lo:hi],
                                    start=(b == 0), stop=(b == 1),
                                    perf_mode=DRSWI)
                        dst = hts[ct // 2][:, 800 * (ct % 2):
                                           800 * (ct % 2) + 800]
                        if nop[0] % 2 == 0:
                            nc.scalar.activation(dst, psH[:], AF.Relu)
                        else:
                            nc.vector.tensor_scalar_max(dst, psH[:], 0.0)
                        nop[0] += 1
                    psO = psOp.tile([128, 800], F32, tag="psO")
                    ti = "qkv".index(t)
                    for pb in range(2):
                        hv = hts[pb][:].rearrange("p (two f) -> p two f",
                                                  two=2)
                        for lo, hi in CH1:
                            nc.tensor.matmul(
                                psO[:, lo:hi],
                                w2s[:, ti * 512 + pb * 256:
                                    ti * 512 + pb * 256 + 256],
                                hv[:, :, lo:hi],
                                start=(pb == 0), stop=(pb == 1),
                                perf_mode=DRSWI)
                    ot = opool.tile([INNER, BN], F16, tag="out")
                    if s == "b" and t == "v":
                        # drain tail: halve copy+DMA latency by splitting
                        # across ACT and DVE with per-half DMA-out
                        nc.scalar.copy(ot[:, 0:400], psO[0:64, 0:400])
                        nc.sync.dma_start(outs[(s, t)][:, 0:400],
                                          ot[:, 0:400])
                        nc.vector.tensor_copy(ot[:, 400:800],
                                              psO[0:64, 400:800])
                        nc.sync.dma_start(outs[(s, t)][:, 400:800],
                                          ot[:, 400:800])
                    else:
                        if nop[0] % 2 == 0:
                            nc.scalar.copy(ot[:], psO[0:64, :])
                        else:
                            nc.vector.tensor_copy(ot[:], psO[0:64, :])
                        nc.sync.dma_start(outs[(s, t)][:], ot[:])
                    nop[0] += 1

    _split_multi_waits(nc)
    return nc


# ---------------------------------------------------------------- prog2

def build_prog2():
    """Attention program, sharded over p (this core's 8 b-batches).

    Inputs (fp8 unless noted):
      ka_dr   [128, 16384]  per q: 256-wide stationary block (i-pad, W1=0)
      qb_dr   [128, 1600]   [qb^T | qb^T], rows 64:128 zero
      va2_dr  [128, 8192]   per j: 256-wide (vaL | vaR) block, /VSCALE
      m1_dr   [128, 384]    path1 selector one-hot cells
      vhat_bT2 [128, 800] bf16   v̂b^T twice (rows 0:64 and 64:128)
      kb_dr   [128, 2048]   per p: 256-wide stationary block
      qa_dr   [128, 12800]  per 1024-chunk c: [qa_c | qa_c] (last 256-wide)
      vb2_dr  [128, 2048]   per p: 256-wide (vbL | vbR) block, /VSCALE
      m8_dr   [128, 464]    path2 selector one-hot cells
      vhat_a2 [128, 3200] bf16   v̂a^T in chunk-pair layout
    Outputs (f32):
      out1 [64, PB]: row q, col p: sum_n cos1
      out2 [128, 4]: row r: g=r//32, s=(r%32)//8, p=r%8;
                     q = 8*(2*g + s//2) + 4*(s%2) + col
    """
    nc = bass.Bass("TRN2", target_bir_lowering=False, debug=False,
                   num_devices=CORES)
    din = {}
    for name, shape, dt in [
        ("ka_dr", [128, 64 * 256], F8), ("qb_dr", [128, 1600], F8),
        ("va2_dr", [128, 32 * 256], F8), ("m1_dr", [128, 384], F8),
        ("vhat_bT2", [128, 800], BF16),
        ("kb_dr", [128, 8 * 256], F8), ("qa_dr", [128, 12800], F8),
        ("vb2_dr", [128, 8 * 256], F8), ("m8_dr", [128, 464], F8),
        ("vhat_a2", [128, 3200], BF16),
    ]:
        din[name] = nc.dram_tensor(name, shape, dt, kind="ExternalInput").ap()
    out1 = nc.dram_tensor("out1", [64, PB], F32, kind="ExternalOutput").ap()
    out2 = nc.dram_tensor("out2", [128, 4], F32, kind="ExternalOutput").ap()
    _register_bias_consts(nc, [EXP_BIAS, EPS * EPS])

    pool1 = _act_sq_spread(64, POOL1)
    pool2 = _act_sq_spread(32, POOL2)
    # path2 units not on Pool: first ACTSQ2 of them skip the SBUF copy and
    # run SQ as an ACT Square from PSUM, M as a DVE mul from PSUM
    nd2 = 32 - POOL2
    actsq2 = _act_sq_spread(nd2, ACTSQ2)

    with tile.TileContext(nc) as tc:
        from contextlib import ExitStack
        with ExitStack() as ctx:
            inp = ctx.enter_context(tc.tile_pool(name="inp", bufs=1))
            sb = {}

            def load(name, cols=None, cname=None):
                ap = din[name]
                if cols is not None:
                    ap = ap[:, cols[0]:cols[1]]
                cname = cname or name
                t = inp.tile(list(ap.shape), ap.dtype, tag=cname,
                             name=f"sb_{cname}")
                nc.sync.dma_start(t[:], ap[:])
                sb[cname] = t

            # path1-critical tensors first so compute starts early
            load("ka_dr", cols=(0, 2048), cname="ka0")
            load("qb_dr")
            load("m1_dr")
            load("va2_dr", cols=(0, 2048), cname="va0")
            load("vhat_bT2")
            for c in range(1, 8):
                load("ka_dr", cols=(2048 * c, 2048 * (c + 1)), cname=f"ka{c}")
                if c < 4:
                    load("va2_dr", cols=(2048 * c, 2048 * (c + 1)),
                         cname=f"va{c}")
            load("kb_dr")
            load("qa_dr")
            load("vb2_dr")
            load("m8_dr")
            load("vhat_a2")

            def ka_blk(q):
                return sb[f"ka{q // 8}"][:, 256 * (q % 8):256 * (q % 8) + 256]

            def va_blk(j):
                return sb[f"va{j // 8}"][:, 256 * (j % 8):256 * (j % 8) + 256]

            epool = ctx.enter_context(tc.tile_pool(name="epool", bufs=E1_BUFS))
            mpool = ctx.enter_context(tc.tile_pool(name="mpool", bufs=MPOOL_BUFS))
            fin = ctx.enter_context(tc.tile_pool(name="fin", bufs=1))

            # ---------------- path 1: per q-pair over this core's (p n) ----
            with (
                tc.tile_pool(name="ps_s1", bufs=2, space="PSUM") as ps_s1,
                tc.tile_pool(name="ps_a1", bufs=2, space="PSUM") as ps_a1,
                tc.tile_pool(name="ps_p1", bufs=1, space="PSUM") as ps_p1,
            ):
                P1h = [ps_p1.tile([128, 400], F32, tag=f"P1{h}",
                                  name=f"P1{h}")
                       for h in range(2)]
                qbv = sb["qb_dr"][:].rearrange("p (two f) -> p two f", two=2)
                pend1 = []
                pendE = []
                pendM = []
                nsel = [0]

                def _sel1(jj, h2, sqm):
                    sqmv = sqm[:].rearrange("p (two f) -> p two f", two=2)
                    nc.tensor.matmul(P1h[h2][:],
                                     sb["m1_dr"][:, 4 * jj:4 * jj + 256],
                                     sqmv,
                                     start=(jj == 0), stop=(jj == 31),
                                     perf_mode=DRSWI,
                                     skip_group_check=True)

                def _align_copy1(jj, E):
                    Ev = E[:].rearrange("p (two f) -> p two f", two=2)
                    for h2, (lo, hi) in enumerate(((0, 400), (400, 800))):
                        As = ps_a1.tile([128, 400], F32, tag="As1")
                        nc.tensor.matmul(As[:], va_blk(jj), Ev[:, :, lo:hi],
                                         start=True, stop=True,
                                         perf_mode=DRSWI)
                        Asb = mpool.tile([128, 400], BF16, tag="Asb")
                        nc.vector.tensor_copy(Asb[:], As[:])
                        pendM.append((jj, h2, Asb))

                def _muls1(jj, h2, Asb):
                    SQM = mpool.tile([128, 800], F8, tag="SQM")
                    vh = sb["vhat_bT2"][:, 400 * h2:400 * h2 + 400]
                    eng = nc.gpsimd if pool1[2 * jj + h2] else nc.vector
                    eng.tensor_mul(SQM[:, 0:400], Asb[:], Asb[:])
                    eng.tensor_mul(SQM[:, 400:800], Asb[:], vh)
                    pend1.append((jj, h2, SQM))
                    if len(pend1) > 2 * SEL_DEPTH:
                        _sel1(*pend1.pop(0))

                for j in range(32):
                    E = epool.tile([128, 1600], F8, tag="E1")
                    for h in range(2):
                        q = 2 * j + h
                        S = ps_s1.tile([128, 800], F32, tag="S1")
                        for lo, hi in CH1:
                            nc.tensor.matmul(S[:, lo:hi], ka_blk(q),
                                             qbv[:, :, lo:hi],
                                             start=True, stop=True,
                                             perf_mode=DRSWI)
                        nc.scalar.activation(E[:, 800 * h:800 * h + 800],
                                             S[:], AF.Exp, scale=0.125,
                                             bias=EXP_BIAS)
                    pendE.append((j, E))
                    if len(pendE) > PENDE_D:
                        _align_copy1(*pendE.pop(0))
                    while len(pendM) > PENDM_D:
                        _muls1(*pendM.pop(0))
                for jj, E in pendE:
                    _align_copy1(jj, E)
                pendE.clear()
                for args in pendM:
                    _muls1(*args)
                pendM.clear()
                for args in pend1:
                    _sel1(*args)
                pend1.clear()

                # epilogue 1: cos1 = dot' * exp(-0.5*ln(ny2' + eps^2))
                r1 = fin.tile([64, PB], F32, tag="r1")
                for h2 in range(2):
                    lg = fin.tile([64, 400], F32, tag=f"lg1{h2}",
                                  name=f"lg1{h2}")
                    nc.scalar.activation(lg[:], P1h[h2][0:64, :], AF.Ln,
                                         bias=EPS * EPS)
                    rc = fin.tile([64, 400], F32, tag=f"rc1{h2}",
                                  name=f"rc1{h2}")
                    nc.scalar.activation(rc[:], lg[:], AF.Exp, scale=-0.5)
                    cos1 = fin.tile([64, 400], F32, tag=f"cos1{h2}",
                                    name=f"cos1{h2}")
                    nc.vector.tensor_mul(cos1[:], P1h[h2][64:128, :], rc[:])
                    nc.vector.tensor_reduce(
                        r1[:, 4 * h2:4 * h2 + 4],
                        cos1[:].rearrange("q (p n) -> q p n", n=N),
                        mybir.AxisListType.X, mybir.AluOpType.add)
                nc.sync.dma_start(out1[:], r1[:])

            # ---------------- path 2: per p over all (q n) -----------------
            with (
                tc.tile_pool(name="ps_s2", bufs=2, space="PSUM") as ps_s2,
                tc.tile_pool(name="ps_a2", bufs=2, space="PSUM") as ps_a2,
                tc.tile_pool(name="ps_p2", bufs=1, space="PSUM") as ps_p2,
            ):
                P2d = ps_p2.tile([128, 400], F32, tag="P2d")
                P2n = ps_p2.tile([128, 400], F32, tag="P2n")
                nd2_i = [0]
                pend2 = []

                def _sel2(r0a, m2, sq2, first, last):
                    nc.tensor.matmul(
                        P2n[:],
                        sb["m8_dr"][:, 2 * r0a:2 * r0a + 256],
                        sq2[:].rearrange("p (two f) -> p two f", two=2),
                        start=first, stop=last, perf_mode=DRSWI,
                        skip_group_check=True)
                    nc.tensor.matmul(
                        P2d[:],
                        sb["m8_dr"][:, 2 * r0a:2 * r0a + 256],
                        m2[:].rearrange("p (two f) -> p two f", two=2),
                        start=first, stop=last, perf_mode=DRSWI,
                        skip_group_check=True)

                u2 = 32
                for p in range(PB):
                    kb = sb["kb_dr"][:, 256 * p:256 * p + 256]
                    E2 = epool.tile([128, 6400], F8, tag="E2", bufs=E2_BUFS)
                    for ci, c0 in enumerate(range(0, 6400, 1024)):
                        w = min(1024, 6400 - c0)
                        S2 = ps_s2.tile([128, 1024], F32, tag="S2")
                        # qa_dr chunk ci holds [qa_c | qa_c], each w wide
                        qoff = 2048 * ci
                        qav = sb["qa_dr"][:, qoff:qoff + 2 * w].rearrange(
                            "p (two f) -> p two f", two=2)
                        for lo in range(0, w, 512):
                            hi = min(lo + 512, w)
                            nc.tensor.matmul(S2[:, lo:hi], kb,
                                             qav[:, :, lo:hi],
                                             start=True, stop=True,
                                             perf_mode=DRSWI)
                        nc.scalar.activation(E2[:, c0:c0 + w], S2[:, 0:w],
                                             AF.Exp, scale=0.125,
                                             bias=EXP_BIAS)
                    vb = sb["vb2_dr"][:, 256 * p:256 * p + 256]
                    for g in range(4):
                        As2 = [ps_a2.tile([128, 400], F32, tag="As2",
                                          name=f"As2{h2}")
                               for h2 in range(2)]
                        for h2 in range(2):
                            j2 = 2 * g + h2
                            e2v = E2[:, 800 * j2:800 * j2 + 800].rearrange(
                                "p (two f) -> p two f", two=2)
                            nc.tensor.matmul(As2[h2][:], vb, e2v,
                                             start=True, stop=True,
                                             perf_mode=DRSWI)
                        M2 = mpool.tile([128, 800], F8, tag="M2")
                        SQ2 = mpool.tile([128, 800], F8, tag="SQ2")
                        vh2 = sb["vhat_a2"][:, 800 * g:800 * g + 800]
                        if pool2[u2 - 32]:
                            As2b = mpool.tile([128, 800], BF16, tag="As2b")
                            for h2 in range(2):
                                nc.vector.tensor_copy(
                                    As2b[:, 400 * h2:400 * h2 + 400],
                                    As2[h2][:])
                            nc.gpsimd.tensor_mul(SQ2[:], As2b[:], As2b[:])
                            nc.gpsimd.tensor_mul(M2[:], As2b[:], vh2)
                        elif actsq2[nd2_i[0] % nd2]:
                            # no SBUF copy: SQ on ACT, M on DVE, both PSUM
                            for h2 in range(2):
                                sl = slice(400 * h2, 400 * h2 + 400)
                                nc.scalar.activation(SQ2[:, sl],
                                                     As2[h2][:], AF.Square)
                                nc.vector.tensor_mul(M2[:, sl], As2[h2][:],
                                                     vh2[:, sl])
                            nd2_i[0] += 1
                        else:
                            As2b = mpool.tile([128, 800], BF16, tag="As2b")
                            for h2 in range(2):
                                nc.vector.tensor_copy(
                                    As2b[:, 400 * h2:400 * h2 + 400],
                                    As2[h2][:])
                            nc.vector.tensor_mul(SQ2[:], As2b[:], As2b[:])
                            nc.vector.tensor_mul(M2[:], As2b[:], vh2)
                            nd2_i[0] += 1
                        u2 += 1
                        r0a = 32 * g + p
                        first = (p == 0 and g == 0)
                        last = (p == PB - 1 and g == 3)
                        pend2.append((r0a, M2, SQ2, first, last))
                        if len(pend2) > SEL_DEPTH:
                            _sel2(*pend2.pop(0))
                for args in pend2:
                    _sel2(*args)
                pend2.clear()

                # epilogue 2
                lg2 = fin.tile([128, 400], F32, tag="lg2")
                nc.scalar.activation(lg2[:], P2n[:], AF.Ln, bias=EPS * EPS)
                rc2 = fin.tile([128, 400], F32, tag="rc2")
                nc.scalar.activation(rc2[:], lg2[:], AF.Exp, scale=-0.5)
                cos2 = fin.tile([128, 400], F32, tag="cos2")
                nc.vector.tensor_mul(cos2[:], P2d[:], rc2[:])
                r2 = fin.tile([128, 4], F32, tag="r2")
                nc.vector.tensor_reduce(
                    r2[:], cos2[:].rearrange("r (g n) -> r g n", n=N),
                    mybir.AxisListType.X, mybir.AluOpType.add)
                nc.sync.dma_start(out2[:], r2[:])

    _split_multi_waits(nc)
    return nc


# ---------------------------------------------------------------- host

_progs = {}


def _install_compile_cache():
    """Persist compiled NEFF-wrapped custom calls across processes: walrus
    compilation takes tens of seconds per program and bass2jax recompiles
    in every fresh process otherwise."""
    import hashlib
    import pathlib
    from concourse import bass2jax
    if getattr(bass2jax, "_ant_disk_cache", False):
        return
    bass2jax._ant_disk_cache = True
    orig = bass2jax.neuronx_cc_hook
    cdir = pathlib.Path(os.environ.get("BASS_NEFF_CACHE",
                                       "/tmp/bass_neff_cache"))
    try:
        cdir.mkdir(parents=True, exist_ok=True)
    except OSError:
        return

    def cached_hook(code, code_format, platform_version, file_prefix):
        try:
            key = hashlib.sha256(
                bytes(code) + b"|" + bytes(code_format)).hexdigest()
            path = cdir / f"{key}.neffcall"
            if path.exists():
                return 0, path.read_bytes()
        except Exception:
            return orig(code, code_format, platform_version, file_prefix)
        rc, blob = orig(code, code_format, platform_version, file_prefix)
        if rc == 0:
            try:
                tmp = path.with_suffix(f".tmp{os.getpid()}")
                tmp.write_bytes(blob)
                tmp.rename(path)
            except OSError:
                pass
        return rc, blob

    bass2jax.neuronx_cc_hook = cached_hook
    try:
        import libneuronxla
        if libneuronxla.neuronx_cc is orig:
            libneuronxla.neuronx_cc = cached_hook
    except ImportError:
        pass


def _get_progs():
    if "p1" not in _progs:
        _install_compile_cache()
        _progs["p1"] = build_prog1()
        _progs["p2"] = build_prog2()
    return _progs["p1"], _progs["p2"]


def _masters():
    """Selector master constants (fp8). Window for target row base r is
    buf[:, 2r : 2r+256]; with DRSwInterleave col->row map row = 127 -
    (Z - 2r)/2 for even cells Z (W0, ifmap half 0) and row = 127 -
    (Z - 1 - 2r)/2 for odd cells (W1, half 1)."""
    m1 = np.zeros((128, 384), NPF8)
    m1[0:64, 254] = 1.0    # W0 (SQ) up-plane -> row q0      (ny2 of q0)
    m1[64:128, 252] = 1.0  # W0 (SQ) down-plane -> row q0+1  (ny2 of q1)
    m1[0:64, 127] = 1.0    # W1 (M) up-plane -> row 64+q0    (dot of q0)
    m1[64:128, 125] = 1.0  # W1 (M) down-plane -> row 65+q0  (dot of q1)
    m8 = np.zeros((128, 464), NPF8)
    m8[0:64, 254] = 1.0    # W0 (j2a) up -> row r0a
    m8[64:128, 238] = 1.0  # W0 (j2a) down -> row r0a+8
    m8[0:64, 223] = 1.0    # W1 (j2b) up -> row r0a+16
    m8[64:128, 207] = 1.0  # W1 (j2b) down -> row r0a+24
    return m1, m8


def _prep1(features_a, features_b, Wq1, Wq2, Wk1, Wk2, Wv1, Wv2):
    """Host prep for prog1: returns per-core input dicts."""
    cc = np.ascontiguousarray
    fa = np.asarray(features_a, np.float32).reshape(B, C, N)
    fb = np.asarray(features_b, np.float32).reshape(B, C, N)

    def f_dr(f8core):
        # [PB, C, N] -> [C, (b n)] = [512, 800] -> [128, (pair, two, 800)]
        x = f8core.transpose(1, 0, 2).reshape(C, BN)
        x = x.reshape(2, 2, 128, BN)           # [pair, two, k, col]
        x = x.transpose(2, 0, 1, 3).reshape(128, 3200)
        return cc(x.astype(NPF8))

    w1_dr = np.zeros((128, 6144), np.float32)
    w2_dr = np.zeros((128, 1536), np.float32)
    for ti, (W1, W2) in enumerate(((Wq1, Wq2), (Wk1, Wk2), (Wv1, Wv2))):
        W1 = np.asarray(W1, np.float32)
        W2 = np.asarray(W2, np.float32)
        for b in range(2):
            for ct in range(4):
                blk = _ilv(W1[256 * b:256 * b + 128, 128 * ct:128 * ct + 128],
                           W1[256 * b + 128:256 * b + 256,
                              128 * ct:128 * ct + 128])
                w1_dr[:, ti * 2048 + b * 1024 + ct * 256:
                      ti * 2048 + b * 1024 + ct * 256 + 256] = blk
            p0 = np.zeros((128, 128), np.float32)
            p1_ = np.zeros((128, 128), np.float32)
            p0[:, 0:64] = W2[256 * b:256 * b + 128, :]
            p1_[:, 0:64] = W2[256 * b + 128:256 * b + 256, :]
            w2_dr[:, ti * 512 + b * 256:ti * 512 + b * 256 + 256] = \
                _ilv(p0, p1_)
    w1_dr = cc(w1_dr.astype(NPF8))
    w2_dr = cc(w2_dr.astype(NPF8))

    return [dict(fa_dr=f_dr(fa[PB * i:PB * (i + 1)]),
                 fb_dr=f_dr(fb[PB * i:PB * (i + 1)]),
                 w1_dr=w1_dr, w2_dr=w2_dr)
            for i in range(CORES)]


def _prep2(qaT, kaT, vaT, qbT, kbT, vbT):
    """Host prep for prog2. qaT/kaT/vaT [64, B*N] f32; qbT/kbT/vbT lists
    of per-core [64, BN] f32."""
    cc = np.ascontiguousarray

    def pad_i(x):  # [64, cols] -> [128, cols] zeros below
        out = np.zeros((128, x.shape[1]), np.float32)
        out[0:64] = x
        return out

    # ka_dr: per q the stationary block (W0 = ka[q] [i, m] col-padded)
    ka_dr = np.zeros((128, 64 * 256), np.float32)
    for q in range(B):
        W0 = np.zeros((128, 128), np.float32)
        W0[0:64, 0:100] = kaT[:, 100 * q:100 * q + 100]
        ka_dr[:, 256 * q:256 * q + 256] = _ilv(W0, np.zeros_like(W0))
    ka_dr = cc(ka_dr.astype(NPF8))

    # qa_dr: per 1024-col chunk [chunk | chunk]
    qa_pad = pad_i(qaT)
    qa_dr = np.zeros((128, 12800), np.float32)
    off = 0
    for c0 in range(0, B * N, 1024):
        w = min(1024, B * N - c0)
        qa_dr[:, off:off + w] = qa_pad[:, c0:c0 + w]
        qa_dr[:, off + w:off + 2 * w] = qa_pad[:, c0:c0 + w]
        off += 2 * w
    qa_dr = cc(qa_dr.astype(NPF8))

    # va2_dr: per j = q-pair, (vaL | vaR) scaled
    va = (vaT.T.reshape(B, N, INNER) / VSCALE).astype(np.float32)
    va2_dr = np.zeros((128, 32 * 256), np.float32)
    for j in range(32):
        W0 = np.zeros((128, 128), np.float32)
        W1 = np.zeros((128, 128), np.float32)
        W0[0:100, 0:64] = va[2 * j]
        W1[0:100, 64:128] = va[2 * j + 1]
        va2_dr[:, 256 * j:256 * j + 256] = _ilv(W0, W1)
    va2_dr = cc(va2_dr.astype(NPF8))

    na = np.maximum(np.sqrt((vaT * vaT).sum(0)), EPS)
    vhat_aT = vaT / na[None, :]
    vhat_a2 = np.zeros((128, B * N // 2), np.float32)
    for j2 in range(8):
        vhat_a2[0:64, 400 * j2:400 * (j2 + 1)] = \
            vhat_aT[:, 800 * j2:800 * j2 + 400]
        vhat_a2[64:128, 400 * j2:400 * (j2 + 1)] = \
            vhat_aT[:, 800 * j2 + 400:800 * (j2 + 1)]
    vhat_a2 = cc(vhat_a2.astype(ml_dtypes.bfloat16))

    m1, m8 = _masters()

    in2 = []
    for i in range(CORES):
        qb_dr = np.zeros((128, 1600), np.float32)
        qb_dr[0:64, 0:800] = qbT[i]
        qb_dr[0:64, 800:1600] = qbT[i]
        qb_dr = cc(qb_dr.astype(NPF8))

        kb_dr = np.zeros((128, 8 * 256), np.float32)
        for p in range(PB):
            W0 = np.zeros((128, 128), np.float32)
            W0[0:64, 0:100] = kbT[i][:, 100 * p:100 * p + 100]
            kb_dr[:, 256 * p:256 * p + 256] = _ilv(W0, np.zeros_like(W0))
        kb_dr = cc(kb_dr.astype(NPF8))

        vb = (vbT[i].T.reshape(PB, N, INNER) / VSCALE).astype(np.float32)
        vb2_dr = np.zeros((128, 8 * 256), np.float32)
        for p in range(PB):
            W0 = np.zeros((128, 128), np.float32)
            W1 = np.zeros((128, 128), np.float32)
            W0[0:100, 0:64] = vb[p]
            W1[0:100, 64:128] = vb[p]
            vb2_dr[:, 256 * p:256 * p + 256] = _ilv(W0, W1)
        vb2_dr = cc(vb2_dr.astype(NPF8))

        nb = np.maximum(np.sqrt((vbT[i] * vbT[i]).sum(0)), EPS)
        vhat_bT = vbT[i] / nb[None, :]
        vhat_bT2 = cc(np.vstack([vhat_bT, vhat_bT]).astype(
            ml_dtypes.bfloat16))

        in2.append(dict(
            ka_dr=ka_dr, qb_dr=qb_dr, va2_dr=va2_dr, m1_dr=m1,
            vhat_bT2=vhat_bT2, kb_dr=kb_dr, qa_dr=qa_dr, vb2_dr=vb2_dr,
            m8_dr=m8, vhat_a2=vhat_a2))
    return in2


def kernel(features_a, features_b, Wq1, Wq2, Wk1, Wk2, Wv1, Wv2):
    nc1, nc2 = _get_progs()

    in1 = _prep1(features_a, features_b, Wq1, Wq2, Wk1, Wk2, Wv1, Wv2)
    res1 = run_bass_kernel_spmd(nc1, in1, core_ids=list(range(CORES)))

    def gather(name):
        return np.concatenate(
            [res1.results[i][name].astype(np.float32)
             for i in range(CORES)], axis=1)

    qaT, kaT, vaT = gather("qaT8"), gather("kaT8"), gather("vaT8")
    qbT = [res1.results[i]["qbT8"].astype(np.float32) for i in range(CORES)]
    kbT = [res1.results[i]["kbT8"].astype(np.float32) for i in range(CORES)]
    vbT = [res1.results[i]["vbT8"].astype(np.float32) for i in range(CORES)]

    in2 = _prep2(qaT, kaT, vaT, qbT, kbT, vbT)
    res2 = run_bass_kernel_spmd(nc2, in2, core_ids=list(range(CORES)))

    sim = np.zeros((B, B), np.float32)
    rr = np.arange(128)
    g = rr // 32
    s_ = (rr % 32) // 8
    p_idx = np.broadcast_to((rr % 8)[:, None], (128, 4))
    q_idx = (8 * (2 * g + s_ // 2) + 4 * (s_ % 2))[:, None] + \
        np.arange(4)[None, :]
    for i in range(CORES):
        o1 = res2.results[i]["out1"]       # [64(q), PB]
        o2 = res2.results[i]["out2"]       # [128, 4]
        blk = o1.T.copy()                  # [PB, 64] path1 sums
        blk[p_idx, q_idx] += o2
        sim[PB * i:PB * (i + 1)] = blk / N
    return sim


# revision 34
# speedup vs baseline: 1.0029x; 1.0029x over previous
"""Trainium2 Bass kernel for nn_AttentionSimilarity.

Contract: kernel(**inputs) takes the FULL unsharded inputs (numpy) and
returns the FULL [64, 64] similarity matrix, distributing work across 8
NeuronCores internally.

Structure:
  prog1 (projections, sharded by batch): each core projects its 8
    a-batches and 8 b-batches through the three two-layer MLPs using
    fp8e4m3 DoubleRowSwInterleave matmuls (two 128-deep contraction
    chunks fused per PE instruction at 0.5 cycles/col), emitting
    q/k/v chunks in [inner, (batch, n)] fp16 layout. Host gathers the
    a-side to full tensors.
  prog2 (attention, sharded by p = b-side batch): per core, both
    attention paths for its 8 p's against all 64 q's. All matmuls are
    fp8 DoubleRowSwInterleave: scores pair (valid, zero) halves;
    aligned pairs the (vaL, vaR) two-plane trick; the dot/norm
    reductions over the inner dim use sliding-window one-hot selector
    weights paired (SQ, M) so one PE pass accumulates both the cosine
    numerator and the norm into a single PSUM accumulator.

Math notes:
  - softmax feeds only cosine similarity, which is scale-invariant in
    the aligned vector, so the softmax max-shift and denominator cancel:
    softmax reduces to exp(scores/8 + bias) for any constant bias.
  - va/vb are pre-scaled by 1/VSCALE so aligned values and their squares
    stay in fp8e4m3 range; the scale cancels exactly in dot/|y|.
  - 1/max(|y|, eps) is computed as exp(-0.5 * ln(ny^2 + eps^2)).

DoubleRowSwInterleave weight packing (validated on HW): for logical
W0, W1 [K=128, M=128], the SBUF buffer holds
  buf[k, 2u] = W0[k, 127-u], buf[k, 2u+1] = W1[k, 127-u]
and out = W0^T @ X0 + W1^T @ X1 where X0/X1 are the (two, f) halves of
the ifmap access pattern. Selector masters slide their 256-col window
by +2 per target row, so 4 fixed one-hot cells serve every window.
"""

import os
import sys

sys.path.insert(0, "/opt/trn_rl_repo")
os.environ.setdefault("NEURON_RT_RESET_CORES", "1")

import numpy as np
import ml_dtypes

import bass_rust
import concourse.bass as bass
import concourse.mybir as mybir
import concourse.tile as tile
from concourse.bass_utils import run_bass_kernel_spmd

F32 = mybir.dt.float32
BF16 = mybir.dt.bfloat16
F16 = mybir.dt.float16
F8 = mybir.dt.float8e4
NPF8 = ml_dtypes.float8_e4m3
AF = mybir.ActivationFunctionType
DRSWI = mybir.MatmulPerfMode.DoubleRowSwInterleave

B = 64          # batches per side
C = 512         # channels
N = 100         # H*W tokens per batch
INNER = 64      # projected dim
CORES = 8
PB = B // CORES  # batches per core (8)
BN = PB * N      # 800: (batch, n) columns per core chunk
EPS = 1e-8
VSCALE = 64.0    # va/vb pre-scale so As and As^2 fit fp8e4m3
EXP_BIAS = float(os.environ.get("K_EXP_BIAS", "-0.25"))
# of path1's 64 half-units, how many run their SQ/M mul pair on Pool
# (DVE otherwise); path2 mul pairs always run on Pool via
# scalar_tensor_tensor (billed at the higher default GPSIMD efficiency).
POOL1 = int(os.environ.get("K_POOL1", "32"))
POOL2 = int(os.environ.get("K_POOL2", "15"))
ACTSQ2 = int(os.environ.get("K_ACTSQ2", "0"))
SEL_DEPTH = int(os.environ.get("K_SEL_DEPTH", "6"))
E1_BUFS = int(os.environ.get("K_E1_BUFS", "4"))
E2_BUFS = int(os.environ.get("K_E2_BUFS", "2"))
PENDE_D = int(os.environ.get("K_PENDE_D", "2"))
PENDM_D = int(os.environ.get("K_PENDM_D", "2"))
MPOOL_BUFS = int(os.environ.get("K_MPOOL_BUFS", "6"))

CH1 = [(0, 512), (512, 800)]

_waitsplit_ctr = [0]


def _split_multi_waits(nc, max_waits=1):
    """This container's walrus build accepts at most ONE sync wait per
    instruction; Tile attaches several. Move extras onto preceding
    same-engine NoOps (engines are in-order, so semantics hold)."""
    n_split = 0
    for f in nc.m.functions:
        for blk in f.blocks:
            insts = list(blk.instructions)
            new_list = []
            changed = False
            for inst in insts:
                si = inst.sync_info
                waits = list(si.on_wait) if (si is not None and si.on_wait) else []
                if len(waits) > max_waits:
                    for w in waits[:-max_waits]:
                        _waitsplit_ctr[0] += 1
                        nop = mybir.InstNoOp(
                            name=f"I-waitsplit-{_waitsplit_ctr[0]}",
                            engine=inst.engine,
                            ins=[],
                            outs=[],
                            sync_info=bass_rust.SyncInfo(on_wait=[w], on_update=[]),
                        )
                        nc.register_instruction(nop, overwrite=True)
                        new_list.append(nop)
                        n_split += 1
                    si.on_wait = waits[-max_waits:]
                    inst.sync_info = si
                    changed = True
                new_list.append(inst)
            if changed:
                blk.instructions = new_list
    return n_split


def _ilv(W0, W1):
    """Pack logical stationary pair [K, M] x2 -> interleaved-reversed
    [K, 2M] f32 buffer for DoubleRowSwInterleave (cast to fp8 later)."""
    K_, M_ = W0.shape
    buf = np.zeros((K_, 2 * M_), np.float32)
    buf[:, 0::2] = W0[:, ::-1]
    buf[:, 1::2] = W1[:, ::-1]
    return buf


def _act_sq_spread(total, n_act):
    """Evenly spread n_act True flags over `total` slots."""
    return [((u * n_act) // total) != (((u + 1) * n_act) // total)
            for u in range(total)]


def _register_bias_consts(nc, values):
    """Activation float biases need a registered [128, 1] const AP."""
    for v in values:
        if (mybir.dt.float32, v) in nc.const_aps.aps:
            continue
        t = nc.alloc_sbuf_tensor(f"const-float32-{v}", [128, 1], F32)
        nc.gpsimd.memset(t.ap(), v)
        nc.const_aps.aps[(mybir.dt.float32, v)] = t.ap()
    nc.all_engine_barrier()


# ---------------------------------------------------------------- prog1

def build_prog1():
    """Projection program (fp8 DR). Per-core inputs:
      fa_dr, fb_dr [128, 3200] f8: features, DR pair layout:
        col 1600*b + 800*two + n  <->  channel 256*b + 128*two, row k
      w1_dr [128, 6144] f8: per (proj t, pair b, couttile ct) a 256-wide
        interleaved stationary block
      w2_dr [128, 1536] f8: per (t, b) a 256-wide block (out cols 64:128
        zero-padded)
    Outputs: qaT8/kaT8/vaT8/qbT8/kbT8/vbT8 [64, BN] f16.
    """
    nc = bass.Bass("TRN2", target_bir_lowering=False, debug=False,
                   num_devices=CORES)
    fa = nc.dram_tensor("fa_dr", [128, 3200], F8, kind="ExternalInput").ap()
    fb = nc.dram_tensor("fb_dr", [128, 3200], F8, kind="ExternalInput").ap()
    w1 = nc.dram_tensor("w1_dr", [128, 6144], F8, kind="ExternalInput").ap()
    w2 = nc.dram_tensor("w2_dr", [128, 1536], F8, kind="ExternalInput").ap()
    outs = {(s, t): nc.dram_tensor(f"{t}{s}T8", [INNER, BN], F16,
                                   kind="ExternalOutput").ap()
            for s in "ab" for t in "qkv"}

    with tile.TileContext(nc) as tc:
        with (
            tc.tile_pool(name="wpool", bufs=1) as wpool,
            tc.tile_pool(name="fpool", bufs=1) as fpool,
            tc.tile_pool(name="hpool", bufs=2) as hpool,
            tc.tile_pool(name="opool", bufs=2) as opool,
            tc.tile_pool(name="psH", bufs=3, space="PSUM") as psHp,
            tc.tile_pool(name="psO", bufs=1, space="PSUM") as psOp,
        ):
            w1q = wpool.tile([128, 2048], F8, tag="w1q", name="w1q")
            nc.sync.dma_start(w1q[:, 0:1024], w1[:, 0:1024])
            fas = fpool.tile([128, 3200], F8, tag="fa", name="fas")
            nc.sync.dma_start(fas[:, 0:1600], fa[:, 0:1600])
            nc.sync.dma_start(w1q[:, 1024:2048], w1[:, 1024:2048])
            nc.sync.dma_start(fas[:, 1600:3200], fa[:, 1600:3200])
            w1kv = wpool.tile([128, 4096], F8, tag="w1kv", name="w1kv")
            nc.sync.dma_start(w1kv[:], w1[:, 2048:6144])
            w2s = wpool.tile([128, 1536], F8, tag="w2", name="w2s")
            nc.sync.dma_start(w2s[:], w2[:])
            fbs = fpool.tile([128, 3200], F8, tag="fb", name="fbs")
            nc.sync.dma_start(fbs[:], fb[:])

            def w1blk(t, b, ct):
                ti = "qkv".index(t)
                off = ti * 2048 + b * 1024 + ct * 256
                if ti == 0:
                    return w1q[:, off:off + 256]
                return w1kv[:, off - 2048:off - 2048 + 256]

            nop = [0]
            for s, feat in (("a", fas), ("b", fbs)):
                fv = feat[:].rearrange("p (b two f) -> p b two f", b=2, two=2)
                for t in "qkv":
                    hts = [hpool.tile([128, 1600], F8, tag=f"h{pb}",
                                      name=f"h{s}{t}{pb}")
                           for pb in range(2)]
                    for ct in range(4):
                        psH = psHp.tile([128, 800], F32, tag="psH")
                        for b in range(2):
                            for lo, hi in CH1:
                                nc.tensor.matmul(
                                    psH[:, lo:hi], w1blk(t, b, ct),
                                    fv[:, b, :, lo:hi],
                                    start=(b == 0), stop=(b == 1),
                                    perf_mode=DRSWI)
                        dst = hts[ct // 2][:, 800 * (ct % 2):
                                           800 * (ct % 2) + 800]
                        if nop[0] % 2 == 0:
                            nc.scalar.activation(dst, psH[:], AF.Relu)
                        else:
                            nc.vector.tensor_scalar_max(dst, psH[:], 0.0)
                        nop[0] += 1
                    psO = psOp.tile([128, 800], F32, tag="psO")
                    ti = "qkv".index(t)
                    for pb in range(2):
                        hv = hts[pb][:].rearrange("p (two f) -> p two f",
                                                  two=2)
                        for lo, hi in CH1:
                            nc.tensor.matmul(
                                psO[:, lo:hi],
                                w2s[:, ti * 512 + pb * 256:
                                    ti * 512 + pb * 256 + 256],
                                hv[:, :, lo:hi],
                                start=(pb == 0), stop=(pb == 1),
                                perf_mode=DRSWI)
                    ot = opool.tile([INNER, BN], F16, tag="out")
                    if nop[0] % 2 == 0:
                        nc.scalar.copy(ot[:], psO[0:64, :])
                    else:
                        nc.vector.tensor_copy(ot[:], psO[0:64, :])
                    nop[0] += 1
                    nc.sync.dma_start(outs[(s, t)][:], ot[:])

    _split_multi_waits(nc)
    return nc


# ---------------------------------------------------------------- prog2

def build_prog2():
    """Attention program, sharded over p (this core's 8 b-batches).

    Inputs (fp8 unless noted):
      ka_dr   [128, 16384]  per q: 256-wide stationary block (i-pad, W1=0)
      qb_dr   [128, 1600]   [qb^T | qb^T], rows 64:128 zero
      va2_dr  [128, 8192]   per j: 256-wide (vaL | vaR) block, /VSCALE
      m1_dr   [128, 384]    path1 selector one-hot cells
      vhat_bT2 [128, 800] bf16   v̂b^T twice (rows 0:64 and 64:128)
      kb_dr   [128, 2048]   per p: 256-wide stationary block
      qa_dr   [128, 12800]  per 1024-chunk c: [qa_c | qa_c] (last 256-wide)
      vb2_dr  [128, 2048]   per p: 256-wide (vbL | vbR) block, /VSCALE
      m8_dr   [128, 464]    path2 selector one-hot cells
      vhat_a2 [128, 3200] bf16   v̂a^T in chunk-pair layout
    Outputs (f32):
      out1 [64, PB]: row q, col p: sum_n cos1
      out2 [128, 4]: row r: g=r//32, s=(r%32)//8, p=r%8;
                     q = 8*(2*g + s//2) + 4*(s%2) + col
    """
    nc = bass.Bass("TRN2", target_bir_lowering=False, debug=False,
                   num_devices=CORES)
    din = {}
    for name, shape, dt in [
        ("ka_dr", [128, 64 * 256], F8), ("qb_dr", [128, 1600], F8),
        ("va2_dr", [128, 32 * 256], F8), ("m1_dr", [128, 384], F8),
        ("vhat_bT2", [128, 800], BF16),
        ("kb_dr", [128, 8 * 256], F8), ("qa_dr", [128, 12800], F8),
        ("vb2_dr", [128, 8 * 256], F8), ("m8_dr", [128, 464], F8),
        ("vhat_a2", [128, 3200], BF16),
    ]:
        din[name] = nc.dram_tensor(name, shape, dt, kind="ExternalInput").ap()
    out1 = nc.dram_tensor("out1", [64, PB], F32, kind="ExternalOutput").ap()
    out2 = nc.dram_tensor("out2", [128, 4], F32, kind="ExternalOutput").ap()
    _register_bias_consts(nc, [EXP_BIAS, EPS * EPS])

    pool1 = _act_sq_spread(64, POOL1)
    pool2 = _act_sq_spread(32, POOL2)
    # path2 units not on Pool: first ACTSQ2 of them skip the SBUF copy and
    # run SQ as an ACT Square from PSUM, M as a DVE mul from PSUM
    nd2 = 32 - POOL2
    actsq2 = _act_sq_spread(nd2, ACTSQ2)

    with tile.TileContext(nc) as tc:
        from contextlib import ExitStack
        with ExitStack() as ctx:
            inp = ctx.enter_context(tc.tile_pool(name="inp", bufs=1))
            sb = {}

            def load(name, cols=None, cname=None):
                ap = din[name]
                if cols is not None:
                    ap = ap[:, cols[0]:cols[1]]
                cname = cname or name
                t = inp.tile(list(ap.shape), ap.dtype, tag=cname,
                             name=f"sb_{cname}")
                nc.sync.dma_start(t[:], ap[:])
                sb[cname] = t

            # path1-critical tensors first so compute starts early
            load("ka_dr", cols=(0, 2048), cname="ka0")
            load("qb_dr")
            load("m1_dr")
            load("va2_dr", cols=(0, 2048), cname="va0")
            load("vhat_bT2")
            for c in range(1, 8):
                load("ka_dr", cols=(2048 * c, 2048 * (c + 1)), cname=f"ka{c}")
                if c < 4:
                    load("va2_dr", cols=(2048 * c, 2048 * (c + 1)),
                         cname=f"va{c}")
            load("kb_dr")
            load("qa_dr")
            load("vb2_dr")
            load("m8_dr")
            load("vhat_a2")

            def ka_blk(q):
                return sb[f"ka{q // 8}"][:, 256 * (q % 8):256 * (q % 8) + 256]

            def va_blk(j):
                return sb[f"va{j // 8}"][:, 256 * (j % 8):256 * (j % 8) + 256]

            epool = ctx.enter_context(tc.tile_pool(name="epool", bufs=E1_BUFS))
            mpool = ctx.enter_context(tc.tile_pool(name="mpool", bufs=MPOOL_BUFS))
            fin = ctx.enter_context(tc.tile_pool(name="fin", bufs=1))

            # ---------------- path 1: per q-pair over this core's (p n) ----
            with (
                tc.tile_pool(name="ps_s1", bufs=2, space="PSUM") as ps_s1,
                tc.tile_pool(name="ps_a1", bufs=2, space="PSUM") as ps_a1,
                tc.tile_pool(name="ps_p1", bufs=1, space="PSUM") as ps_p1,
            ):
                P1h = [ps_p1.tile([128, 400], F32, tag=f"P1{h}",
                                  name=f"P1{h}")
                       for h in range(2)]
                qbv = sb["qb_dr"][:].rearrange("p (two f) -> p two f", two=2)
                pend1 = []
                pendE = []
                pendM = []
                nsel = [0]

                def _sel1(jj, h2, sqm):
                    sqmv = sqm[:].rearrange("p (two f) -> p two f", two=2)
                    nc.tensor.matmul(P1h[h2][:],
                                     sb["m1_dr"][:, 4 * jj:4 * jj + 256],
                                     sqmv,
                                     start=(jj == 0), stop=(jj == 31),
                                     perf_mode=DRSWI,
                                     skip_group_check=True)

                def _align_copy1(jj, E):
                    Ev = E[:].rearrange("p (two f) -> p two f", two=2)
                    for h2, (lo, hi) in enumerate(((0, 400), (400, 800))):
                        As = ps_a1.tile([128, 400], F32, tag="As1")
                        nc.tensor.matmul(As[:], va_blk(jj), Ev[:, :, lo:hi],
                                         start=True, stop=True,
                                         perf_mode=DRSWI)
                        Asb = mpool.tile([128, 400], BF16, tag="Asb")
                        nc.vector.tensor_copy(Asb[:], As[:])
                        pendM.append((jj, h2, Asb))

                def _muls1(jj, h2, Asb):
                    SQM = mpool.tile([128, 800], F8, tag="SQM")
                    vh = sb["vhat_bT2"][:, 400 * h2:400 * h2 + 400]
                    eng = nc.gpsimd if pool1[2 * jj + h2] else nc.vector
                    eng.tensor_mul(SQM[:, 0:400], Asb[:], Asb[:])
                    eng.tensor_mul(SQM[:, 400:800], Asb[:], vh)
                    pend1.append((jj, h2, SQM))
                    if len(pend1) > 2 * SEL_DEPTH:
                        _sel1(*pend1.pop(0))

                for j in range(32):
                    E = epool.tile([128, 1600], F8, tag="E1")
                    for h in range(2):
                        q = 2 * j + h
                        S = ps_s1.tile([128, 800], F32, tag="S1")
                        for lo, hi in CH1:
                            nc.tensor.matmul(S[:, lo:hi], ka_blk(q),
                                             qbv[:, :, lo:hi],
                                             start=True, stop=True,
                                             perf_mode=DRSWI)
                        nc.scalar.activation(E[:, 800 * h:800 * h + 800],
                                             S[:], AF.Exp, scale=0.125,
                                             bias=EXP_BIAS)
                    pendE.append((j, E))
                    if len(pendE) > PENDE_D:
                        _align_copy1(*pendE.pop(0))
                    while len(pendM) > PENDM_D:
                        _muls1(*pendM.pop(0))
                for jj, E in pendE:
                    _align_copy1(jj, E)
                pendE.clear()
                for args in pendM:
                    _muls1(*args)
                pendM.clear()
                for args in pend1:
                    _sel1(*args)
                pend1.clear()

                # epilogue 1: cos1 = dot' * exp(-0.5*ln(ny2' + eps^2))
                r1 = fin.tile([64, PB], F32, tag="r1")
                for h2 in range(2):
                    lg = fin.tile([64, 400], F32, tag=f"lg1{h2}",
                                  name=f"lg1{h2}")
                    nc.scalar.activation(lg[:], P1h[h2][0:64, :], AF.Ln,
                                         bias=EPS * EPS)
                    rc = fin.tile([64, 400], F32, tag=f"rc1{h2}",
                                  name=f"rc1{h2}")
                    nc.scalar.activation(rc[:], lg[:], AF.Exp, scale=-0.5)
                    cos1 = fin.tile([64, 400], F32, tag=f"cos1{h2}",
                                    name=f"cos1{h2}")
                    nc.vector.tensor_mul(cos1[:], P1h[h2][64:128, :], rc[:])
                    nc.vector.tensor_reduce(
                        r1[:, 4 * h2:4 * h2 + 4],
                        cos1[:].rearrange("q (p n) -> q p n", n=N),
                        mybir.AxisListType.X, mybir.AluOpType.add)
                nc.sync.dma_start(out1[:], r1[:])

            # ---------------- path 2: per p over all (q n) -----------------
            with (
                tc.tile_pool(name="ps_s2", bufs=2, space="PSUM") as ps_s2,
                tc.tile_pool(name="ps_a2", bufs=2, space="PSUM") as ps_a2,
                tc.tile_pool(name="ps_p2", bufs=1, space="PSUM") as ps_p2,
            ):
                P2d = ps_p2.tile([128, 400], F32, tag="P2d")
                P2n = ps_p2.tile([128, 400], F32, tag="P2n")
                nd2_i = [0]
                pend2 = []

                def _sel2(r0a, m2, sq2, first, last):
                    nc.tensor.matmul(
                        P2n[:],
                        sb["m8_dr"][:, 2 * r0a:2 * r0a + 256],
                        sq2[:].rearrange("p (two f) -> p two f", two=2),
                        start=first, stop=last, perf_mode=DRSWI,
                        skip_group_check=True)
                    nc.tensor.matmul(
                        P2d[:],
                        sb["m8_dr"][:, 2 * r0a:2 * r0a + 256],
                        m2[:].rearrange("p (two f) -> p two f", two=2),
                        start=first, stop=last, perf_mode=DRSWI,
                        skip_group_check=True)

                u2 = 32
                for p in range(PB):
                    kb = sb["kb_dr"][:, 256 * p:256 * p + 256]
                    E2 = epool.tile([128, 6400], F8, tag="E2", bufs=E2_BUFS)
                    for ci, c0 in enumerate(range(0, 6400, 1024)):
                        w = min(1024, 6400 - c0)
                        S2 = ps_s2.tile([128, 1024], F32, tag="S2")
                        # qa_dr chunk ci holds [qa_c | qa_c], each w wide
                        qoff = 2048 * ci
                        qav = sb["qa_dr"][:, qoff:qoff + 2 * w].rearrange(
                            "p (two f) -> p two f", two=2)
                        for lo in range(0, w, 512):
                            hi = min(lo + 512, w)
                            nc.tensor.matmul(S2[:, lo:hi], kb,
                                             qav[:, :, lo:hi],
                                             start=True, stop=True,
                                             perf_mode=DRSWI)
                        nc.scalar.activation(E2[:, c0:c0 + w], S2[:, 0:w],
                                             AF.Exp, scale=0.125,
                                             bias=EXP_BIAS)
                    vb = sb["vb2_dr"][:, 256 * p:256 * p + 256]
                    for g in range(4):
                        As2 = [ps_a2.tile([128, 400], F32, tag="As2",
                                          name=f"As2{h2}")
                               for h2 in range(2)]
                        for h2 in range(2):
                            j2 = 2 * g + h2
                            e2v = E2[:, 800 * j2:800 * j2 + 800].rearrange(
                                "p (two f) -> p two f", two=2)
                            nc.tensor.matmul(As2[h2][:], vb, e2v,
                                             start=True, stop=True,
                                             perf_mode=DRSWI)
                        M2 = mpool.tile([128, 800], F8, tag="M2")
                        SQ2 = mpool.tile([128, 800], F8, tag="SQ2")
                        vh2 = sb["vhat_a2"][:, 800 * g:800 * g + 800]
                        if pool2[u2 - 32]:
                            As2b = mpool.tile([128, 800], BF16, tag="As2b")
                            for h2 in range(2):
                                nc.vector.tensor_copy(
                                    As2b[:, 400 * h2:400 * h2 + 400],
                                    As2[h2][:])
                            nc.gpsimd.tensor_mul(SQ2[:], As2b[:], As2b[:])
                            nc.gpsimd.tensor_mul(M2[:], As2b[:], vh2)
                        elif actsq2[nd2_i[0] % nd2]:
                            # no SBUF copy: SQ on ACT, M on DVE, both PSUM
                            for h2 in range(2):
                                sl = slice(400 * h2, 400 * h2 + 400)
                                nc.scalar.activation(SQ2[:, sl],
                                                     As2[h2][:], AF.Square)
                                nc.vector.tensor_mul(M2[:, sl], As2[h2][:],
                                                     vh2[:, sl])
                            nd2_i[0] += 1
                        else:
                            As2b = mpool.tile([128, 800], BF16, tag="As2b")
                            for h2 in range(2):
                                nc.vector.tensor_copy(
                                    As2b[:, 400 * h2:400 * h2 + 400],
                                    As2[h2][:])
                            nc.vector.tensor_mul(SQ2[:], As2b[:], As2b[:])
                            nc.vector.tensor_mul(M2[:], As2b[:], vh2)
                            nd2_i[0] += 1
                        u2 += 1
                        r0a = 32 * g + p
                        first = (p == 0 and g == 0)
                        last = (p == PB - 1 and g == 3)
                        pend2.append((r0a, M2, SQ2, first, last))
                        if len(pend2) > SEL_DEPTH:
                            _sel2(*pend2.pop(0))
                for args in pend2:
                    _sel2(*args)
                pend2.clear()

                # epilogue 2
                lg2 = fin.tile([128, 400], F32, tag="lg2")
                nc.scalar.activation(lg2[:], P2n[:], AF.Ln, bias=EPS * EPS)
                rc2 = fin.tile([128, 400], F32, tag="rc2")
                nc.scalar.activation(rc2[:], lg2[:], AF.Exp, scale=-0.5)
                cos2 = fin.tile([128, 400], F32, tag="cos2")
                nc.vector.tensor_mul(cos2[:], P2d[:], rc2[:])
                r2 = fin.tile([128, 4], F32, tag="r2")
                nc.vector.tensor_reduce(
                    r2[:], cos2[:].rearrange("r (g n) -> r g n", n=N),
                    mybir.AxisListType.X, mybir.AluOpType.add)
                nc.sync.dma_start(out2[:], r2[:])

    _split_multi_waits(nc)
    return nc


# ---------------------------------------------------------------- host

_progs = {}


def _install_compile_cache():
    """Persist compiled NEFF-wrapped custom calls across processes: walrus
    compilation takes tens of seconds per program and bass2jax recompiles
    in every fresh process otherwise."""
    import hashlib
    import pathlib
    from concourse import bass2jax
    if getattr(bass2jax, "_ant_disk_cache", False):
        return
    bass2jax._ant_disk_cache = True
    orig = bass2jax.neuronx_cc_hook
    cdir = pathlib.Path(os.environ.get("BASS_NEFF_CACHE",
                                       "/tmp/bass_neff_cache"))
    try:
        cdir.mkdir(parents=True, exist_ok=True)
    except OSError:
        return

    def cached_hook(code, code_format, platform_version, file_prefix):
        try:
            key = hashlib.sha256(
                bytes(code) + b"|" + bytes(code_format)).hexdigest()
            path = cdir / f"{key}.neffcall"
            if path.exists():
                return 0, path.read_bytes()
        except Exception:
            return orig(code, code_format, platform_version, file_prefix)
        rc, blob = orig(code, code_format, platform_version, file_prefix)
        if rc == 0:
            try:
                tmp = path.with_suffix(f".tmp{os.getpid()}")
                tmp.write_bytes(blob)
                tmp.rename(path)
            except OSError:
                pass
        return rc, blob

    bass2jax.neuronx_cc_hook = cached_hook
    try:
        import libneuronxla
        if libneuronxla.neuronx_cc is orig:
            libneuronxla.neuronx_cc = cached_hook
    except ImportError:
        pass


def _get_progs():
    if "p1" not in _progs:
        _install_compile_cache()
        _progs["p1"] = build_prog1()
        _progs["p2"] = build_prog2()
    return _progs["p1"], _progs["p2"]


def _masters():
    """Selector master constants (fp8). Window for target row base r is
    buf[:, 2r : 2r+256]; with DRSwInterleave col->row map row = 127 -
    (Z - 2r)/2 for even cells Z (W0, ifmap half 0) and row = 127 -
    (Z - 1 - 2r)/2 for odd cells (W1, half 1)."""
    m1 = np.zeros((128, 384), NPF8)
    m1[0:64, 254] = 1.0    # W0 (SQ) up-plane -> row q0      (ny2 of q0)
    m1[64:128, 252] = 1.0  # W0 (SQ) down-plane -> row q0+1  (ny2 of q1)
    m1[0:64, 127] = 1.0    # W1 (M) up-plane -> row 64+q0    (dot of q0)
    m1[64:128, 125] = 1.0  # W1 (M) down-plane -> row 65+q0  (dot of q1)
    m8 = np.zeros((128, 464), NPF8)
    m8[0:64, 254] = 1.0    # W0 (j2a) up -> row r0a
    m8[64:128, 238] = 1.0  # W0 (j2a) down -> row r0a+8
    m8[0:64, 223] = 1.0    # W1 (j2b) up -> row r0a+16
    m8[64:128, 207] = 1.0  # W1 (j2b) down -> row r0a+24
    return m1, m8


def _prep1(features_a, features_b, Wq1, Wq2, Wk1, Wk2, Wv1, Wv2):
    """Host prep for prog1: returns per-core input dicts."""
    cc = np.ascontiguousarray
    fa = np.asarray(features_a, np.float32).reshape(B, C, N)
    fb = np.asarray(features_b, np.float32).reshape(B, C, N)

    def f_dr(f8core):
        # [PB, C, N] -> [C, (b n)] = [512, 800] -> [128, (pair, two, 800)]
        x = f8core.transpose(1, 0, 2).reshape(C, BN)
        x = x.reshape(2, 2, 128, BN)           # [pair, two, k, col]
        x = x.transpose(2, 0, 1, 3).reshape(128, 3200)
        return cc(x.astype(NPF8))

    w1_dr = np.zeros((128, 6144), np.float32)
    w2_dr = np.zeros((128, 1536), np.float32)
    for ti, (W1, W2) in enumerate(((Wq1, Wq2), (Wk1, Wk2), (Wv1, Wv2))):
        W1 = np.asarray(W1, np.float32)
        W2 = np.asarray(W2, np.float32)
        for b in range(2):
            for ct in range(4):
                blk = _ilv(W1[256 * b:256 * b + 128, 128 * ct:128 * ct + 128],
                           W1[256 * b + 128:256 * b + 256,
                              128 * ct:128 * ct + 128])
                w1_dr[:, ti * 2048 + b * 1024 + ct * 256:
                      ti * 2048 + b * 1024 + ct * 256 + 256] = blk
            p0 = np.zeros((128, 128), np.float32)
            p1_ = np.zeros((128, 128), np.float32)
            p0[:, 0:64] = W2[256 * b:256 * b + 128, :]
            p1_[:, 0:64] = W2[256 * b + 128:256 * b + 256, :]
            w2_dr[:, ti * 512 + b * 256:ti * 512 + b * 256 + 256] = \
                _ilv(p0, p1_)
    w1_dr = cc(w1_dr.astype(NPF8))
    w2_dr = cc(w2_dr.astype(NPF8))

    return [dict(fa_dr=f_dr(fa[PB * i:PB * (i + 1)]),
                 fb_dr=f_dr(fb[PB * i:PB * (i + 1)]),
                 w1_dr=w1_dr, w2_dr=w2_dr)
            for i in range(CORES)]


def _prep2(qaT, kaT, vaT, qbT, kbT, vbT):
    """Host prep for prog2. qaT/kaT/vaT [64, B*N] f32; qbT/kbT/vbT lists
    of per-core [64, BN] f32."""
    cc = np.ascontiguousarray

    def pad_i(x):  # [64, cols] -> [128, cols] zeros below
        out = np.zeros((128, x.shape[1]), np.float32)
        out[0:64] = x
        return out

    # ka_dr: per q the stationary block (W0 = ka[q] [i, m] col-padded)
    ka_dr = np.zeros((128, 64 * 256), np.float32)
    for q in range(B):
        W0 = np.zeros((128, 128), np.float32)
        W0[0:64, 0:100] = kaT[:, 100 * q:100 * q + 100]
        ka_dr[:, 256 * q:256 * q + 256] = _ilv(W0, np.zeros_like(W0))
    ka_dr = cc(ka_dr.astype(NPF8))

    # qa_dr: per 1024-col chunk [chunk | chunk]
    qa_pad = pad_i(qaT)
    qa_dr = np.zeros((128, 12800), np.float32)
    off = 0
    for c0 in range(0, B * N, 1024):
        w = min(1024, B * N - c0)
        qa_dr[:, off:off + w] = qa_pad[:, c0:c0 + w]
        qa_dr[:, off + w:off + 2 * w] = qa_pad[:, c0:c0 + w]
        off += 2 * w
    qa_dr = cc(qa_dr.astype(NPF8))

    # va2_dr: per j = q-pair, (vaL | vaR) scaled
    va = (vaT.T.reshape(B, N, INNER) / VSCALE).astype(np.float32)
    va2_dr = np.zeros((128, 32 * 256), np.float32)
    for j in range(32):
        W0 = np.zeros((128, 128), np.float32)
        W1 = np.zeros((128, 128), np.float32)
        W0[0:100, 0:64] = va[2 * j]
        W1[0:100, 64:128] = va[2 * j + 1]
        va2_dr[:, 256 * j:256 * j + 256] = _ilv(W0, W1)
    va2_dr = cc(va2_dr.astype(NPF8))

    na = np.maximum(np.sqrt((vaT * vaT).sum(0)), EPS)
    vhat_aT = vaT / na[None, :]
    vhat_a2 = np.zeros((128, B * N // 2), np.float32)
    for j2 in range(8):
        vhat_a2[0:64, 400 * j2:400 * (j2 + 1)] = \
            vhat_aT[:, 800 * j2:800 * j2 + 400]
        vhat_a2[64:128, 400 * j2:400 * (j2 + 1)] = \
            vhat_aT[:, 800 * j2 + 400:800 * (j2 + 1)]
    vhat_a2 = cc(vhat_a2.astype(ml_dtypes.bfloat16))

    m1, m8 = _masters()

    in2 = []
    for i in range(CORES):
        qb_dr = np.zeros((128, 1600), np.float32)
        qb_dr[0:64, 0:800] = qbT[i]
        qb_dr[0:64, 800:1600] = qbT[i]
        qb_dr = cc(qb_dr.astype(NPF8))

        kb_dr = np.zeros((128, 8 * 256), np.float32)
        for p in range(PB):
            W0 = np.zeros((128, 128), np.float32)
            W0[0:64, 0:100] = kbT[i][:, 100 * p:100 * p + 100]
            kb_dr[:, 256 * p:256 * p + 256] = _ilv(W0, np.zeros_like(W0))
        kb_dr = cc(kb_dr.astype(NPF8))

        vb = (vbT[i].T.reshape(PB, N, INNER) / VSCALE).astype(np.float32)
        vb2_dr = np.zeros((128, 8 * 256), np.float32)
        for p in range(PB):
            W0 = np.zeros((128, 128), np.float32)
            W1 = np.zeros((128, 128), np.float32)
            W0[0:100, 0:64] = vb[p]
            W1[0:100, 64:128] = vb[p]
            vb2_dr[:, 256 * p:256 * p + 256] = _ilv(W0, W1)
        vb2_dr = cc(vb2_dr.astype(NPF8))

        nb = np.maximum(np.sqrt((vbT[i] * vbT[i]).sum(0)), EPS)
        vhat_bT = vbT[i] / nb[None, :]
        vhat_bT2 = cc(np.vstack([vhat_bT, vhat_bT]).astype(
            ml_dtypes.bfloat16))

        in2.append(dict(
            ka_dr=ka_dr, qb_dr=qb_dr, va2_dr=va2_dr, m1_dr=m1,
            vhat_bT2=vhat_bT2, kb_dr=kb_dr, qa_dr=qa_dr, vb2_dr=vb2_dr,
            m8_dr=m8, vhat_a2=vhat_a2))
    return in2


def kernel(features_a, features_b, Wq1, Wq2, Wk1, Wk2, Wv1, Wv2):
    nc1, nc2 = _get_progs()

    in1 = _prep1(features_a, features_b, Wq1, Wq2, Wk1, Wk2, Wv1, Wv2)
    res1 = run_bass_kernel_spmd(nc1, in1, core_ids=list(range(CORES)))

    def gather(name):
        return np.concatenate(
            [res1.results[i][name].astype(np.float32)
             for i in range(CORES)], axis=1)

    qaT, kaT, vaT = gather("qaT8"), gather("kaT8"), gather("vaT8")
    qbT = [res1.results[i]["qbT8"].astype(np.float32) for i in range(CORES)]
    kbT = [res1.results[i]["kbT8"].astype(np.float32) for i in range(CORES)]
    vbT = [res1.results[i]["vbT8"].astype(np.float32) for i in range(CORES)]

    in2 = _prep2(qaT, kaT, vaT, qbT, kbT, vbT)
    res2 = run_bass_kernel_spmd(nc2, in2, core_ids=list(range(CORES)))

    sim = np.zeros((B, B), np.float32)
    rr = np.arange(128)
    g = rr // 32
    s_ = (rr % 32) // 8
    p_idx = np.broadcast_to((rr % 8)[:, None], (128, 4))
    q_idx = (8 * (2 * g + s_ // 2) + 4 * (s_ % 2))[:, None] + \
        np.arange(4)[None, :]
    for i in range(CORES):
        o1 = res2.results[i]["out1"]       # [64(q), PB]
        o2 = res2.results[i]["out2"]       # [128, 4]
        blk = o1.T.copy()                  # [PB, 64] path1 sums
        blk[p_idx, q_idx] += o2
        sim[PB * i:PB * (i + 1)] = blk / N
    return sim
